# revision 1
# baseline (speedup 1.0000x reference)
"""Attention-LSTM captioning model on 8 trn2 cores (8-way tensor parallel).

Gate/itr/vocab output dims sharded across cores (full B=64 per core);
attention batch-sharded (8 batches/core, selected via per-core one-hot bsel
data, since the SPMD program is identical on every core). Activations are
transposed [feature, batch]. Per step: AllGather(att_res), AllGather(nh
chunk). Sigmoid(x) is computed as (tanh(x/2)+1)/2 so phase 1 only needs the
{tanh, exp} ACT table; the hidden state is stored as 2*h with h-consuming
weights pre-halved on the host. log_softmax runs in phase 2 (one stats AG).
"""
import numpy as np
import ml_dtypes

import concourse.bacc as bacc
import concourse.mybir as mybir
import concourse.tile as tile
from concourse.ap import AP
from concourse.bass_utils import run_bass_kernel_spmd

BF16_NP = ml_dtypes.bfloat16
FP32 = mybir.dt.float32
BF16 = mybir.dt.bfloat16
AF = mybir.ActivationFunctionType
ALU = mybir.AluOpType
AX = mybir.AxisListType

B, T, R, H, F, E, L, V1 = 64, 20, 1024, 512, 2048, 300, 196, 12001
NC = 8
BMY = B // NC
GC = R // NC              # 128
NGATE = 5 * GC            # 640
VP = 1504
LP = 208
NG = LP // 16             # 13
EP = 384
HCN = H // 128            # 4
FCN = F // 128            # 16
RCN = R // 128            # 8
NBL = BMY * L             # 1568


def _bf(x):
    return np.ascontiguousarray(np.asarray(x, dtype=np.float32)).astype(BF16_NP)


def bcast_free(ap, n):
    """Append a step-0 free dim of size n to an AP (broadcast)."""
    return AP(ap.tensor, ap.offset, list(ap.ap) + [[0, n]])


def host_prep(inputs):
    seq = np.asarray(inputs["seq"])
    att = np.asarray(inputs["att_feats"], dtype=np.float32)
    embed_w = np.asarray(inputs["embed_w"], dtype=np.float32)
    ctx2att_w = np.asarray(inputs["ctx2att_w"], dtype=np.float32)
    ctx2att_b = np.asarray(inputs["ctx2att_b"], dtype=np.float32)
    h2att_w = np.asarray(inputs["h2att_w"], dtype=np.float32)
    h2att_b = np.asarray(inputs["h2att_b"], dtype=np.float32)
    alpha_w = np.asarray(inputs["alpha_w"], dtype=np.float32)
    i2h_w = np.asarray(inputs["i2h_w"], dtype=np.float32)
    i2h_b = np.asarray(inputs["i2h_b"], dtype=np.float32)
    h2h_w = np.asarray(inputs["h2h_w"], dtype=np.float32)
    h2h_b = np.asarray(inputs["h2h_b"], dtype=np.float32)
    a2c_w = np.asarray(inputs["a2c_w"], dtype=np.float32)
    a2c_b = np.asarray(inputs["a2c_b"], dtype=np.float32)
    logit_w = np.asarray(inputs["logit_w"], dtype=np.float32)
    logit_b = np.asarray(inputs["logit_b"], dtype=np.float32)

    xt = embed_w[seq]                                    # [B, T, E]
    xtT = np.zeros((EP, T * B), dtype=np.float32)
    xtT[:E] = xt.transpose(2, 1, 0).reshape(E, T * B)
    xtT[E] = 1.0
    xtT = _bf(xtT)
    bias_gate = i2h_b + h2h_b

    in_maps = []
    for c in range(NC):
        m = {"xtT": xtT}
        grows = np.concatenate([np.arange(gg * R + c * GC, gg * R + (c + 1) * GC)
                                for gg in range(5)])
        i2hT = np.zeros((EP, NGATE), dtype=np.float32)
        i2hT[:E] = i2h_w[grows, :].T
        i2hT[E] = bias_gate[grows]
        m["i2hT"] = _bf(i2hT)
        m["h2hT"] = _bf(h2h_w[grows, :].T * 0.5)
        m["h2attT"] = _bf(h2att_w.T * 0.5)
        m["h2att_bias"] = _bf(h2att_b[None, :])
        m["ctxT"] = _bf(ctx2att_w.T)
        m["ctx_bias"] = _bf(ctx2att_b[None, :])
        amy = att[c * BMY:(c + 1) * BMY]                 # [8, L, F]
        m["attT"] = _bf(amy.transpose(2, 0, 1).reshape(F, NBL))
        alb = np.zeros((NG * 16, 8, F), dtype=np.float32)
        alb[:L] = amy.transpose(1, 0, 2)
        m["att_lb"] = _bf(alb.reshape(NG * 128, F))
        ac = np.zeros((HCN * 128, 64), dtype=np.float32)
        for b in range(BMY):
            ac[:, b * 8 + b] = alpha_w[0]
        m["alpha_cols"] = _bf(ac)
        arows = np.concatenate([np.arange(c * GC, (c + 1) * GC),
                                np.arange(R + c * GC, R + (c + 1) * GC)])
        m["a2cT"] = _bf(a2c_w[arows, :].T)
        m["a2c_bias"] = _bf(a2c_b[arows][None, :])
        vrows = np.arange(c * VP, (c + 1) * VP)
        lw = np.zeros((R, VP), dtype=np.float32)
        lb = np.full((1, VP), -1e30, dtype=np.float32)
        valid = vrows < V1
        lw[:, valid] = logit_w[vrows[valid], :].T * 0.5
        lb[0, valid] = logit_b[vrows[valid]]
        m["logitT"] = _bf(lw)
        m["logit_bias"] = lb
        m["ident"] = _bf(np.eye(128))
        bsel = np.zeros((B, BMY), dtype=np.float32)
        for j in range(BMY):
            bsel[c * BMY + j, j] = 1.0
        m["bsel"] = _bf(bsel)
        in_maps.append(m)
    return in_maps


def build(t_steps=T, probes=(), reps=1, no_cc=False):
    assert t_steps % 2 == 0
    nc = bacc.Bacc("TRN2", target_bir_lowering=False, debug=False,
                   num_devices=NC)
    probes = set(probes)
    NT = t_steps * B // 128
    RG = [list(range(NC))]

    def din(name, shape, dt=BF16):
        return nc.dram_tensor(name, shape, dt, kind="ExternalInput")

    xtT_d = din("xtT", [EP, T * B])
    i2hT_d = din("i2hT", [EP, NGATE])
    h2hT_d = din("h2hT", [R, NGATE])
    h2attT_d = din("h2attT", [R, H])
    h2att_b_d = din("h2att_bias", [1, H])
    ctxT_d = din("ctxT", [F, H])
    ctx_b_d = din("ctx_bias", [1, H])
    attT_d = din("attT", [F, NBL])
    att_lb_d = din("att_lb", [NG * 128, F])
    alpha_d = din("alpha_cols", [HCN * 128, 64])
    a2cT_d = din("a2cT", [F, 256])
    a2c_b_d = din("a2c_bias", [1, 256])
    logitT_d = din("logitT", [R, VP])
    logit_b_d = din("logit_bias", [1, VP], FP32)
    ident_d = din("ident", [128, 128])
    bsel_d = din("bsel", [B, BMY])

    out_d = nc.dram_tensor("logp", [t_steps * B, VP], FP32,
                           kind="ExternalOutput")
    scratch = nc.dram_tensor("logits_scratch", [t_steps * B, VP], FP32)
    agA_out_r = [[nc.dram_tensor(f"agA_out_{rp}_{t}", [B, F], BF16,
                                 addr_space="Shared") for t in range(t_steps)]
                 for rp in range(reps)]
    agH_out_r = [[nc.dram_tensor(f"agH_out_{rp}_{t}", [R, B], BF16,
                                 addr_space="Shared") for t in range(t_steps)]
                 for rp in range(reps)]
    agS_out_r = [nc.dram_tensor(f"agS_out_{rp}", [NC * 128, 2 * NT], FP32,
                                addr_space="Shared") for rp in range(reps)]

    with tile.TileContext(nc) as tc:
        with (
            tc.tile_pool(name="wpool", bufs=1) as wpool,
            tc.tile_pool(name="hpool", bufs=4) as hpool,
            tc.tile_pool(name="psum", bufs=1, space="PSUM") as psum,
            tc.tile_pool(name="dram", bufs=4, space="DRAM") as dpool,
        ):
            def probe_(name, src_ap, shape, dt):
                pd = nc.dram_tensor(f"probe_{name}", list(shape), dt,
                                    kind="ExternalOutput")
                nc.sync.dma_start(out=pd[:], in_=src_ap)

            def load_chunks(pool, dram, cols, n, tag, dt=BF16):
                ts = []
                for i in range(n):
                    t_ = pool.tile([128, cols], dt, tag=f"{tag}{i}",
                                   name=f"{tag}{i}")
                    nc.sync.dma_start(out=t_[:],
                                      in_=dram[i * 128:(i + 1) * 128, :])
                    ts.append(t_)
                return ts

            logitT_s = load_chunks(wpool, logitT_d, VP, RCN, "logitT")
            logit_b_s = wpool.tile([128, VP], FP32, tag="logitb",
                                   name="logitb")
            _lb_src = AP(logit_b_d[:].tensor, logit_b_d[:].offset,
                         [[0, 128], [1, VP]])
            nc.sync.dma_start(out=logit_b_s[:], in_=_lb_src)
            ident_s = wpool.tile([128, 128], BF16, tag="ident", name="ident")
            nc.sync.dma_start(out=ident_s[:], in_=ident_d[:])
            ones64 = wpool.tile([1, B], BF16, tag="ones64", name="ones64")
            nc.vector.memset(ones64[:], 1.0)
            negm_all = wpool.tile([128, NT], FP32, tag="negm_all",
                                  name="negm_all")
            s_all = wpool.tile([128, NT], FP32, tag="s_all", name="s_all")

            with tc.tile_pool(name="w1pool", bufs=1) as w1pool:
                xtT_s = load_chunks(w1pool, xtT_d, T * B, 3, "xtT")
                i2hT_s = load_chunks(w1pool, i2hT_d, NGATE, 3, "i2hT")
                h2hT_s = load_chunks(w1pool, h2hT_d, NGATE, RCN, "h2hT")
                h2attT_s = load_chunks(w1pool, h2attT_d, H, RCN, "h2attT")
                att_lb_s = load_chunks(w1pool, att_lb_d, F, NG, "attlb")
                alpha_s = load_chunks(w1pool, alpha_d, 64, HCN, "alpha")
                a2cT_s = load_chunks(w1pool, a2cT_d, 256, FCN, "a2cT")
                bsel_s = w1pool.tile([B, BMY], BF16, tag="bsel", name="bsel")
                nc.sync.dma_start(out=bsel_s[:], in_=bsel_d[:])
                h2att_b_s = w1pool.tile([1, H], BF16, tag="h2attb",
                                        name="h2attb")
                nc.sync.dma_start(out=h2att_b_s[:], in_=h2att_b_d[:])
                ctx_b_s = w1pool.tile([1, H], BF16, tag="ctxb", name="ctxb")
                nc.sync.dma_start(out=ctx_b_s[:], in_=ctx_b_d[:])
                a2c_b_s = w1pool.tile([1, 256], BF16, tag="a2cb", name="a2cb")
                nc.sync.dma_start(out=a2c_b_s[:], in_=a2c_b_d[:])
                onesNBL = w1pool.tile([1, NBL], BF16, tag="onesNBL",
                                      name="onesNBL")
                nc.vector.memset(onesNBL[:], 1.0)
                p_attT = [w1pool.tile([128, NBL], BF16, tag=f"pattT{hc}",
                                      name=f"pattT{hc}")
                          for hc in range(HCN)]
                stat_all = w1pool.tile([128, NG * 8], BF16, tag="stat_all",
                                       name="stat_all")
                nc.vector.memset(stat_all[:], 0.0)
                w_bf = w1pool.tile([BMY, LP], BF16, tag="w_bf", name="w_bf")
                nc.vector.memset(w_bf[:], 0.0)
                c_st = w1pool.tile([B, GC], FP32, tag="c_st", name="c_st")

                def emit_rep(rep):
                    agA_out = agA_out_r[rep]
                    agH_out = agH_out_r[rep]
                    agS_out = agS_out_r[rep]

                    def probe(name, src_ap, shape, dt):
                        if rep == 0 and name in probes:
                            probe_(name, src_ap, shape, dt)

                    nc.vector.memset(c_st[:], 0.0)
                    hT = hpool.tile([128, RCN * 64], BF16, tag="hT",
                                    name="hT0")
                    nc.vector.memset(hT[:], 0.0)
                    hT_hist = [hT]

                    # ---------- phase 0 ----------
                    with (
                        tc.tile_pool(name=f"ctxpool{rep}", bufs=1) as ctxpool,
                        tc.tile_pool(name=f"stream{rep}", bufs=3) as stream,
                    ):
                        ctxT_s = load_chunks(ctxpool, ctxT_d, H, FCN, "ctxT")
                        QW = 392
                        for q in range(4):
                            n0 = q * QW
                            _pa_tags = ["sums", "ah", "ar", "lg"]
                            pa_ps = [psum.tile([128, QW], FP32,
                                               tag=_pa_tags[hc],
                                               name=f"pa{hc}", bufs=1)
                                     for hc in range(HCN)]
                            for fc in range(FCN):
                                at = stream.tile([128, QW], BF16, tag="attTq",
                                                 name="attTq")
                                nc.sync.dma_start(
                                    out=at[:],
                                    in_=attT_d[fc * 128:(fc + 1) * 128,
                                               n0:n0 + QW])
                                for hc in range(HCN):
                                    nc.tensor.matmul(
                                        pa_ps[hc][:],
                                        ctxT_s[fc][:,
                                                   hc * 128:(hc + 1) * 128],
                                        at[:], start=(fc == 0), stop=False)
                            for hc in range(HCN):
                                nc.tensor.matmul(
                                    pa_ps[hc][:],
                                    ctx_b_s[:, hc * 128:(hc + 1) * 128],
                                    onesNBL[:, n0:n0 + QW], start=False,
                                    stop=True)
                                nc.vector.tensor_copy(
                                    p_attT[hc][:, n0:n0 + QW], pa_ps[hc][:])
                    probe("p_attT0", p_attT[0][:], [128, NBL], BF16)

                    # ---------- phase 1 ----------
                    with tc.tile_pool(name=f"work1_{rep}", bufs=1) as work:
                        for t in range(t_steps):
                            sums_ps = psum.tile([B, NGATE], FP32, tag="sums",
                                                name="sums", bufs=1)
                            for c0 in (0, 512):
                                c1 = min(NGATE, c0 + 512)
                                for kc in range(3):
                                    nc.tensor.matmul(
                                        sums_ps[:, c0:c1],
                                        xtT_s[kc][:, t * B:(t + 1) * B],
                                        i2hT_s[kc][:, c0:c1],
                                        start=(kc == 0), stop=False)
                                for rc in range(RCN):
                                    nc.tensor.matmul(
                                        sums_ps[:, c0:c1],
                                        hT[:, rc * 64:(rc + 1) * 64],
                                        h2hT_s[rc][:, c0:c1],
                                        start=False, stop=(rc == RCN - 1))

                            ah_ps = psum.tile([B, H], FP32, tag="ah",
                                              name="ah", bufs=1)
                            for rc in range(RCN):
                                nc.tensor.matmul(
                                    ah_ps[:], hT[:, rc * 64:(rc + 1) * 64],
                                    h2attT_s[rc][:], start=(rc == 0),
                                    stop=False)
                            nc.tensor.matmul(ah_ps[:], ones64[:],
                                             h2att_b_s[:], start=False,
                                             stop=True)
                            ah_sb = work.tile([B, H], BF16, tag="ah_sb",
                                              name="ah_sb", bufs=1)
                            nc.vector.tensor_copy(ah_sb[:], ah_ps[:])
                            ahT_ps = psum.tile([128, HCN * 8], FP32,
                                               tag="small", name="ahT_ps",
                                               bufs=1)
                            for hc in range(HCN):
                                nc.tensor.matmul(
                                    ahT_ps[:, hc * 8:(hc + 1) * 8],
                                    ah_sb[:, hc * 128:(hc + 1) * 128],
                                    bsel_s[:], start=True, stop=True)
                            ahT = work.tile([128, HCN * 8], BF16,
                                            tag="ahT_sb", name="ahT_sb",
                                            bufs=1)
                            nc.vector.tensor_copy(ahT[:], ahT_ps[:])

                            e_ps = psum.tile([BMY, L], FP32, tag="small",
                                             name="e_ps", bufs=1)
                            for hc in range(HCN):
                                dp = work.tile([128, NBL], BF16, tag="dp",
                                               name="dp", bufs=2)
                                nc.vector.tensor_tensor(
                                    dp[:].rearrange("p (b l) -> p b l",
                                                    b=BMY),
                                    p_attT[hc][:].rearrange(
                                        "p (b l) -> p b l", b=BMY),
                                    bcast_free(ahT[:, hc * 8:(hc + 1) * 8],
                                               L),
                                    op=ALU.add)
                                dt_ = work.tile([128, NBL], BF16, tag="dt",
                                                name="dt", bufs=2)
                                nc.scalar.activation(dt_[:], dp[:], AF.Tanh)
                                for b in range(BMY):
                                    nc.tensor.matmul(
                                        e_ps[:],
                                        alpha_s[hc][:, b * 8:(b + 1) * 8],
                                        dt_[:, b * L:(b + 1) * L],
                                        start=(hc == 0 and b == 0),
                                        stop=(hc == HCN - 1 and
                                              b == BMY - 1))
                                if t == 0 and hc == 0:
                                    probe("dotT0", dt_[:], [128, NBL], BF16)

                            negm = work.tile([BMY, 1], FP32, tag="negm",
                                             name="negm", bufs=1)
                            nc.vector.tensor_reduce(negm[:], e_ps[:],
                                                    axis=AX.X, op=ALU.max,
                                                    negate=True)
                            u = work.tile([BMY, L], FP32, tag="u", name="u",
                                          bufs=1)
                            nc.scalar.activation(u[:], e_ps[:], AF.Exp,
                                                 bias=negm[:])
                            ssum = work.tile([BMY, 1], FP32, tag="ssum",
                                             name="ssum", bufs=1)
                            nc.vector.tensor_reduce(ssum[:], u[:], axis=AX.X,
                                                    op=ALU.add)
                            rinv = work.tile([BMY, 1], FP32, tag="rinv",
                                             name="rinv", bufs=1)
                            nc.vector.reciprocal(rinv[:], ssum[:])
                            nc.vector.tensor_scalar(w_bf[:, 0:L], u[:],
                                                    rinv[:], None,
                                                    op0=ALU.mult)

                            wdr = dpool.tile([BMY, LP], BF16, tag="wdr",
                                             name="wdr")
                            nc.sync.dma_start(out=wdr[:], in_=w_bf[:])
                            for b in range(BMY):
                                nc.sync.dma_start(
                                    out=stat_all[b:128:8, b:NG * 8:8],
                                    in_=wdr[b:b + 1, :].rearrange(
                                        "o (g lp) -> (o lp) g", g=NG))

                            ar_sb = work.tile([BMY, F], BF16, tag="ar_sb",
                                              name="ar_sb", bufs=1)
                            for half in range(2):
                                f0 = half * 1024
                                ar_ps = psum.tile([BMY, 1024], FP32,
                                                  tag="ar", name="ar_ps",
                                                  bufs=1)
                                for g in range(NG):
                                    for qf in range(2):
                                        nc.tensor.matmul(
                                            ar_ps[:,
                                                  qf * 512:(qf + 1) * 512],
                                            stat_all[:, g * 8:(g + 1) * 8],
                                            att_lb_s[g][:,
                                                        f0 + qf * 512:
                                                        f0 + (qf + 1) * 512],
                                            start=(g == 0),
                                            stop=(g == NG - 1))
                                nc.vector.tensor_copy(
                                    ar_sb[:, f0:f0 + 1024], ar_ps[:])
                            agA_in = dpool.tile([BMY, F], BF16, tag="agA_in",
                                                name="agA_in")
                            nc.sync.dma_start(out=agA_in[:], in_=ar_sb[:])
                            if no_cc:
                                nc.sync.dma_start(out=agA_out[t][0:BMY, :],
                                                  in_=agA_in[:])
                            else:
                                nc.gpsimd.collective_compute(
                                    "AllGather", ALU.bypass, replica_groups=RG,
                                    ins=[agA_in.opt()], outs=[agA_out[t][:]])
                            arg_sb = work.tile([B, F], BF16, tag="arg_sb",
                                               name="arg_sb", bufs=1)
                            nc.sync.dma_start(out=arg_sb[:],
                                              in_=agA_out[t][:])
                            arT = work.tile([128, FCN * 64], BF16, tag="arT",
                                            name="arT", bufs=1)
                            for fc in range(FCN):
                                art_ps = psum.tile(
                                    [128, 64], BF16,
                                    tag=("small" if fc % 2 else "ctx"),
                                    name="art_ps", bufs=1)
                                nc.tensor.transpose(
                                    art_ps[:],
                                    arg_sb[:, fc * 128:(fc + 1) * 128],
                                    ident_s[0:B, 0:B])
                                nc.vector.tensor_copy(
                                    arT[:, fc * 64:(fc + 1) * 64],
                                    art_ps[:])

                            ctx_ps = psum.tile([B, 256], FP32, tag="ctx",
                                               name="ctx_ps", bufs=1)
                            for fc in range(FCN):
                                nc.tensor.matmul(
                                    ctx_ps[:], arT[:, fc * 64:(fc + 1) * 64],
                                    a2cT_s[fc][:], start=(fc == 0),
                                    stop=False)
                            nc.tensor.matmul(ctx_ps[:], ones64[:],
                                             a2c_b_s[:], start=False,
                                             stop=True)

                            sig3 = work.tile([B, 384], FP32, tag="sig3",
                                             name="sig3", bufs=1)
                            nc.scalar.activation(sig3[:], sums_ps[:, 0:384],
                                                 AF.Tanh, scale=0.5)
                            sitr = work.tile([B, 256], FP32, tag="sitr",
                                             name="sitr", bufs=1)
                            nc.vector.tensor_copy(sitr[:],
                                                  sums_ps[:, 384:640])
                            itr1 = work.tile([B, GC], FP32, tag="itr1",
                                             name="itr1", bufs=1)
                            nc.vector.tensor_tensor(itr1[:], sitr[:, 0:128],
                                                    ctx_ps[:, 0:128],
                                                    op=ALU.add)
                            itr2 = work.tile([B, GC], FP32, tag="itr2",
                                             name="itr2", bufs=1)
                            nc.vector.tensor_tensor(itr2[:],
                                                    sitr[:, 128:256],
                                                    ctx_ps[:, 128:256],
                                                    op=ALU.add)
                            g_t = work.tile([B, GC], FP32, tag="g_t",
                                            name="g_t", bufs=1)
                            nc.vector.tensor_tensor(g_t[:], itr1[:],
                                                    itr2[:], op=ALU.max)
                            a_t = work.tile([B, GC], FP32, tag="a_t",
                                            name="a_t", bufs=1)
                            nc.vector.scalar_tensor_tensor(
                                a_t[:], sig3[:, 128:256], 1.0, c_st[:],
                                op0=ALU.add, op1=ALU.mult)
                            b_t = work.tile([B, GC], FP32, tag="b_t",
                                            name="b_t", bufs=1)
                            nc.vector.scalar_tensor_tensor(
                                b_t[:], sig3[:, 0:128], 1.0, g_t[:],
                                op0=ALU.add, op1=ALU.mult)
                            nc2_t = work.tile([B, GC], FP32, tag="nc2",
                                              name="nc2", bufs=1)
                            nc.vector.tensor_tensor(nc2_t[:], a_t[:],
                                                    b_t[:], op=ALU.add)
                            nc.vector.tensor_scalar(c_st[:], nc2_t[:], 0.5,
                                                    None, op0=ALU.mult)
                            tnc = work.tile([B, GC], FP32, tag="tnc",
                                            name="tnc", bufs=1)
                            nc.scalar.activation(tnc[:], nc2_t[:], AF.Tanh,
                                                 scale=0.5)
                            nh2 = work.tile([B, GC], BF16, tag="nh2",
                                            name="nh2", bufs=1)
                            nc.vector.scalar_tensor_tensor(
                                nh2[:], sig3[:, 256:384], 1.0, tnc[:],
                                op0=ALU.add, op1=ALU.mult)

                            nhT_ps = psum.tile([GC, B], BF16, tag="small",
                                               name="nhT_ps", bufs=1)
                            nc.tensor.transpose(nhT_ps[:], nh2[:],
                                                ident_s[0:B, 0:B])
                            nhT_sb = work.tile([GC, B], BF16, tag="nhT_sb",
                                               name="nhT_sb", bufs=1)
                            nc.vector.tensor_copy(nhT_sb[:], nhT_ps[:])
                            agH_in = dpool.tile([GC, B], BF16, tag="agH_in",
                                                name="agH_in")
                            nc.sync.dma_start(out=agH_in[:], in_=nhT_sb[:])
                            if no_cc:
                                nc.sync.dma_start(out=agH_out[t][0:GC, :],
                                                  in_=agH_in[:])
                            else:
                                nc.gpsimd.collective_compute(
                                    "AllGather", ALU.bypass, replica_groups=RG,
                                    ins=[agH_in.opt()], outs=[agH_out[t][:]])
                            hT_new = hpool.tile([128, RCN * 64], BF16,
                                                tag="hT", name="hT_new")
                            nc.sync.dma_start(
                                out=hT_new[:].rearrange(
                                    "rl (rc b) -> rl rc b", rc=RCN),
                                in_=agH_out[t][:].rearrange(
                                    "(rc rl) b -> rl rc b", rc=RCN))
                            hT_hist.append(hT_new)
                            hT = hT_new

                            if t == 0:
                                probe("ah0", ah_sb[:], [B, H], BF16)
                                probe("ahT0", ahT[:], [128, HCN * 8], BF16)
                                probe("u0", u[:], [BMY, L], FP32)
                                probe("w0", w_bf[:], [BMY, LP], BF16)
                                probe("statall0", stat_all[:],
                                      [128, NG * 8], BF16)
                                probe("ar0", ar_sb[:], [BMY, F], BF16)
                                probe("arT0", arT[:], [128, FCN * 64], BF16)
                                probe("nh20", nh2[:], [B, GC], BF16)
                                probe("nhT0", nhT_sb[:], [GC, B], BF16)
                                probe("agH0", agH_out[0][:], [R, B], BF16)
                                probe("hT1", hT_new[:], [128, RCN * 64],
                                      BF16)
                                if rep == 0 and "sums0" in probes:
                                    sums_cp = work.tile([B, NGATE], FP32,
                                                        tag="sums_cp",
                                                        name="sums_cp",
                                                        bufs=1)
                                    nc.vector.tensor_copy(sums_cp[:],
                                                          sums_ps[:])
                                    probe("sums0", sums_cp[:], [B, NGATE],
                                          FP32)
                                if rep == 0 and "ctx0" in probes:
                                    ctx_cp = work.tile([B, 256], FP32,
                                                       tag="ctx_cp",
                                                       name="ctx_cp",
                                                       bufs=1)
                                    nc.vector.tensor_copy(ctx_cp[:],
                                                          ctx_ps[:])
                                    probe("ctx0", ctx_cp[:], [B, 256], FP32)
                                probe("sig30", sig3[:], [B, 384], FP32)

                            if t % 2 == 1:
                                lg_sb = work.tile([128, VP], FP32,
                                                  tag="lg_sb", name="lg_sb",
                                                  bufs=1)
                                for c0 in (0, 512, 1024):
                                    c1 = min(VP, c0 + 512)
                                    lg_ps = psum.tile([128, 512], FP32,
                                                      tag="lg", name="lg_ps",
                                                      bufs=1)
                                    for rc in range(RCN):
                                        nc.tensor.matmul(
                                            lg_ps[0:64, 0:c1 - c0],
                                            hT_hist[t][:,
                                                       rc * 64:(rc + 1) * 64],
                                            logitT_s[rc][:, c0:c1],
                                            start=(rc == 0),
                                            stop=(rc == RCN - 1),
                                            tile_position=(0, 0))
                                    for rc in range(RCN):
                                        nc.tensor.matmul(
                                            lg_ps[64:128, 0:c1 - c0],
                                            hT_hist[t + 1][:,
                                                           rc * 64:
                                                           (rc + 1) * 64],
                                            logitT_s[rc][:, c0:c1],
                                            start=(rc == 0),
                                            stop=(rc == RCN - 1),
                                            tile_position=(0, 64))
                                    nc.vector.scalar_tensor_tensor(
                                        lg_sb[:, c0:c1], lg_ps[:, 0:c1 - c0],
                                        1.0, logit_b_s[:, c0:c1],
                                        op0=ALU.mult, op1=ALU.add)
                                nc.sync.dma_start(
                                    out=scratch[(t - 1) * B:(t + 1) * B, :],
                                    in_=lg_sb[:])
                                if t == 1:
                                    probe("lg0", lg_sb[:], [128, VP], FP32)

                    # ---------- phase 2 ----------
                    with tc.tile_pool(name=f"p2_{rep}", bufs=2) as p2:
                        for tt in range(NT):
                            lg = p2.tile([128, VP], FP32, tag="p2lg",
                                         name="p2lg")
                            nc.sync.dma_start(
                                out=lg[:],
                                in_=scratch[tt * 128:(tt + 1) * 128, :])
                            nc.vector.tensor_reduce(
                                negm_all[:, tt:tt + 1], lg[:], axis=AX.X,
                                op=ALU.max, negate=True)
                            junk = p2.tile([128, VP], BF16, tag="p2junk",
                                           name="p2junk")
                            nc.scalar.activation(
                                junk[:], lg[:], AF.Exp,
                                bias=negm_all[:, tt:tt + 1],
                                accum_out=s_all[:, tt:tt + 1])
                        agS_in = dpool.tile([128, 2 * NT], FP32,
                                            tag="agS_in", name="agS_in")
                        nc.sync.dma_start(out=agS_in[:, 0:NT],
                                          in_=negm_all[:])
                        nc.sync.dma_start(out=agS_in[:, NT:2 * NT],
                                          in_=s_all[:])
                        if no_cc:
                            nc.sync.dma_start(out=agS_out[0:128, :],
                                              in_=agS_in[:])
                        else:
                            nc.gpsimd.collective_compute(
                                "AllGather", ALU.bypass, replica_groups=RG,
                                ins=[agS_in.opt()], outs=[agS_out[:]])
                        statg = p2.tile([128, NC * 2 * NT], FP32,
                                        tag="statg", name="statg", bufs=1)
                        nc.sync.dma_start(
                            out=statg[:].rearrange("p (r s) -> p r s", r=NC),
                            in_=agS_out[:].rearrange("(r p) s -> p r s",
                                                     r=NC))
                        sview = statg[:].rearrange("p (r s) -> p s r", r=NC)
                        negM = p2.tile([128, NT], FP32, tag="negM",
                                       name="negM", bufs=1)
                        nc.vector.tensor_reduce(negM[:], sview[:, 0:NT, :],
                                                axis=AX.X, op=ALU.min)
                        earg = p2.tile([128, NT * NC], FP32, tag="earg",
                                       name="earg", bufs=1)
                        nc.vector.tensor_tensor(
                            earg[:].rearrange("p (s r) -> p s r", r=NC),
                            bcast_free(negM[:], NC), sview[:, 0:NT, :],
                            op=ALU.subtract)
                        em = p2.tile([128, NT * NC], FP32, tag="em",
                                     name="em", bufs=1)
                        nc.scalar.activation(em[:], earg[:], AF.Exp)
                        sexp = p2.tile([128, NT * NC], FP32, tag="sexp",
                                       name="sexp", bufs=1)
                        nc.vector.tensor_tensor(
                            sexp[:].rearrange("p (s r) -> p s r", r=NC),
                            em[:].rearrange("p (s r) -> p s r", r=NC),
                            sview[:, NT:2 * NT, :], op=ALU.mult)
                        S_t = p2.tile([128, NT], FP32, tag="S_t",
                                      name="S_t", bufs=1)
                        nc.vector.tensor_reduce(
                            S_t[:],
                            sexp[:].rearrange("p (s r) -> p s r", r=NC),
                            axis=AX.X, op=ALU.add)
                        lnS = p2.tile([128, NT], FP32, tag="lnS",
                                      name="lnS", bufs=1)
                        nc.scalar.activation(lnS[:], S_t[:], AF.Ln)
                        logZ = p2.tile([128, NT], FP32, tag="logZ",
                                       name="logZ", bufs=1)
                        nc.vector.scalar_tensor_tensor(
                            logZ[:], negM[:], -1.0, lnS[:], op0=ALU.mult,
                            op1=ALU.add)
                        probe("logZ", logZ[:], [128, NT], FP32)
                        for tt in range(NT):
                            lg2 = p2.tile([128, VP], FP32, tag="p2lg2",
                                          name="p2lg2")
                            nc.sync.dma_start(
                                out=lg2[:],
                                in_=scratch[tt * 128:(tt + 1) * 128, :])
                            lp_t = p2.tile([128, VP], FP32, tag="p2lp",
                                           name="p2lp")
                            nc.vector.tensor_scalar(
                                lp_t[:], lg2[:], logZ[:, tt:tt + 1], None,
                                op0=ALU.subtract)
                            nc.sync.dma_start(
                                out=out_d[tt * 128:(tt + 1) * 128, :],
                                in_=lp_t[:])

                for rep in range(reps):
                    emit_rep(rep)

    nc.compile()
    return nc, sorted(probes)



_NC_CACHE = {}


def kernel(**inputs):
    """Full-input entry point: returns logp [B, T, V1] float32."""
    from concourse.bass_utils import run_bass_kernel_spmd
    in_maps = host_prep(inputs)
    if "nc" not in _NC_CACHE:
        _NC_CACHE["nc"], _ = build(T, (), reps=1)
    nc = _NC_CACHE["nc"]
    res = run_bass_kernel_spmd(nc, in_maps, list(range(NC)))
    outs = [res.results[c]["logp"] for c in range(NC)]
    full = np.concatenate(outs, axis=1)[:, :V1]          # [T*B, V1]
    logp = full.reshape(T, B, V1).transpose(1, 0, 2)
    return np.ascontiguousarray(logp.astype(np.float32))

